# revision 49
# baseline (speedup 1.0000x reference)
"""Trainium2 Bass kernel for a 2-layer relational GraphSAGE VGAE encoder.

Contract: kernel(**inputs) takes the FULL unsharded inputs (as produced by
setup_inputs()) and returns the full (mu, logvar) tuple.

Strategy (8 NeuronCores, SPMD single NEFF):
  - Nodes block-sharded: core c owns nodes [c*2500, (c+1)*2500), padded to 2560.
  - Edges partitioned by destination-node owner and sorted by destination so
    each <=128-edge chunk scatters into a single 64-node window: the one-hot
    matmuls run at N=64 instead of N=512 (4-8x less PE time than group-wide
    one-hots).
  - Gathers are batched 4 chunks per indirect DMA ([128, 4] index columns)
    to amortize the SWDGE fixed cost.
  - Feature-major [channels, nodes] fp16 everywhere, fp32 PSUM accumulation.
  - BatchNorm (eval) folded into the layer-2 weights on the host; layer 2 has
    a single stacked projection weight [2560 -> 1024] covering mu_l|lv_l
    (aggregated side) and mu_r|lv_r (self side).
  - The aggregated-side projections are PE-transposed to node-major and
    AllGather'd (fp16) per node group so the collectives overlap phase 1;
    the self-side matmuls (phase 3a) cover the last AllGather's tail.
  - Dense layers restructured mc-outer so h accumulates in 1-bank PSUM tiles;
    aggregation for cell j+1 is emitted ahead of dense for cell j so the PE
    never waits on PSUM->SBUF evacuations (which are split across the Vector
    and Scalar engines).
"""
import sys

sys.path.insert(0, "/opt/trn_rl_repo")

import numpy as np

NCORES = 8
N = 20000
E = 100000
IN = 512
HID = 512
CAT = 2560
OUT = 256
BN_EPS = 1e-5

NLOC = N // NCORES          # 2500
NPAD = 2560                 # 20 * 128, 5 * 512
NG = NPAD // 512            # 5 node groups of 512 per core
NREL = 5
P = 128
TABROWS = NCORES * NPAD     # 20480

WIN = 64                    # scatter window width (nodes per one-hot matmul)
WPG = 512 // WIN            # windows per group (8)
NW = NPAD // WIN            # windows per core (40)


# ----------------------------------------------------------------------------
# Host-side preprocessing: sharding, edge chunking, weight folding
# ----------------------------------------------------------------------------

def _chunk_edges_banded(key, ncells, src_vals, col, val, width):
    """Group edges by per-core cell, sort by destination column within the
    cell, and cut into 128-edge chunks.  Each chunk scatters into a narrow
    column band [off, off+W) shared across cores (compile-time constants).
    Bands are forced contiguous and to cover [0, width) so every PSUM
    element gets written.

    Returns: nch [ncells], base [ncells], Ctot, offs [Ctot], ws [Ctot],
             avb [Ctot+1] av column base per chunk,
             idxT [NCORES, 128, Ctot] int32,
             avP [NCORES, 128, avb[-1]] f16 (partition-major one-hot values)
    """
    counts = np.bincount(key, minlength=NCORES * ncells).reshape(NCORES, ncells)
    nch = np.maximum((counts + P - 1) // P, 1).max(axis=0)  # [ncells]
    base = np.concatenate([[0], np.cumsum(nch)[:-1]])
    Ctot = int(nch.sum())

    order = np.lexsort((col, key))
    ks = key[order]
    cols = col[order]
    first_of_run = np.r_[True, ks[1:] != ks[:-1]]
    run_starts = np.flatnonzero(first_of_run)
    run_id = np.cumsum(first_of_run) - 1
    pos = np.arange(len(ks)) - run_starts[run_id]

    core_s = ks // ncells
    cell_s = ks % ncells
    chunk_s = base[cell_s] + pos // P
    row_s = pos % P

    # per-chunk min/max destination column across cores
    lo = np.full(Ctot, width, np.int64)
    hi = np.zeros(Ctot, np.int64)
    np.minimum.at(lo, chunk_s, cols)
    np.maximum.at(hi, chunk_s, cols + 1)
    offs = np.zeros(Ctot, np.int64)
    ws = np.zeros(Ctot, np.int64)
    for cell in range(ncells):
        b, n = base[cell], nch[cell]
        end = 0
        for i in range(b, b + n):
            offs[i] = 0 if i == b else min(lo[i], end)
            end = max(hi[i], offs[i] + 1)
            ws[i] = end - offs[i]
        ws[b + n - 1] = width - offs[b + n - 1]
    avb = np.concatenate([[0], np.cumsum(ws)])

    idxT = np.zeros((NCORES, P, Ctot), np.int32)
    avP = np.zeros((NCORES, P, int(avb[-1])), np.float16)
    idxT[core_s, row_s, chunk_s] = src_vals[order]
    avP[core_s, row_s, avb[chunk_s] + cols - offs[chunk_s]] = val[order]
    return nch, base, Ctot, offs, ws, avb, idxT, avP


def _chunk_edges(key, ncells, src_vals, col, val):
    """Group edges by per-core cell, chunk each cell into 128-edge chunks.

    key: [E] int = core * ncells + cell   (cell < ncells)
    src_vals: [E] int32 gather row index for each edge
    col: [E] int in [0, WIN) one-hot column (dst position within window)
    val: [E] f32 one-hot value (1/cnt)

    Returns: nch [ncells] shared chunk counts (max over cores, >=1),
             base [ncells] chunk base offsets, Ctot,
             idxT [NCORES, 128, Ctot] int32,
             avP [NCORES, 128, Ctot*WIN] f16 (partition-major one-hot values)
    """
    counts = np.bincount(key, minlength=NCORES * ncells).reshape(NCORES, ncells)
    nch = np.maximum((counts + P - 1) // P, 1).max(axis=0)  # [ncells]
    base = np.concatenate([[0], np.cumsum(nch)[:-1]])
    Ctot = int(nch.sum())

    order = np.argsort(key, kind="stable")
    ks = key[order]
    first_of_run = np.r_[True, ks[1:] != ks[:-1]]
    run_starts = np.flatnonzero(first_of_run)
    run_id = np.cumsum(first_of_run) - 1
    pos = np.arange(len(ks)) - run_starts[run_id]

    core_s = ks // ncells
    cell_s = ks % ncells
    chunk_s = base[cell_s] + pos // P
    row_s = pos % P

    idxT = np.zeros((NCORES, P, Ctot), np.int32)
    av = np.zeros((NCORES, Ctot, P, WIN), np.float16)
    idxT[core_s, row_s, chunk_s] = src_vals[order]
    av[core_s, chunk_s, row_s, col[order]] = val[order]
    avP = np.ascontiguousarray(
        av.transpose(0, 2, 1, 3).reshape(NCORES, P, Ctot * WIN))
    return nch, base, Ctot, idxT, avP


def _preprocess(x, edge_index, edge_attr, Wl5, Wr5, bl5,
                Wmu_l, Wmu_r, bmu, Wlv_l, Wlv_r, blv,
                gamma, beta, run_mean, run_var):
    x = np.asarray(x, np.float32)
    src = np.asarray(edge_index[0], np.int64)
    dst = np.asarray(edge_index[1], np.int64)
    rel = np.asarray(edge_attr, np.int64)

    # --- per-node inverse degree ---
    cnt1 = np.bincount(rel * N + dst, minlength=NREL * N).reshape(NREL, N)
    inv1 = 1.0 / np.maximum(cnt1, 1.0)
    cnt2 = np.bincount(dst, minlength=N)
    inv2 = 1.0 / np.maximum(cnt2, 1.0)

    core = dst // NLOC
    loc = dst % NLOC
    g = loc // 512
    w_in_g = (loc // WIN) % WPG
    col = loc % WIN

    # layer-1 cells ordered (group, rel): banded chunks sorted by dst column
    cell1 = g * NREL + rel
    key1 = core * (NG * NREL) + cell1
    nch1, base1, C1, offs1, ws1, avb1, a1i, a1v = _chunk_edges_banded(
        key1, NG * NREL, src.astype(np.int32), loc % 512, inv1[rel, dst], 512)

    # layer-2 cells ordered (group, window); gather rows from the all-gathered
    # table whose layout is [g][core][col]: row = g*8*512 + core*512 + col.
    src_loc = src % NLOC
    tabrow = ((src_loc // 512) * (NCORES * 512) + (src // NLOC) * 512
              + src_loc % 512).astype(np.int32)
    # split by source group: rows < 16384 are in table A (groups 0-3,
    # complete after AG3); group-4 rows live in table B (after AG4)
    isB = tabrow >= (NG - 1) * NCORES * 512
    selA, selB = ~isB, isB
    key2 = core * NG + g
    nch2, base2, C2, offs2, ws2, avb2, a2i, a2v = _chunk_edges_banded(
        key2[selA], NG, tabrow[selA], (loc % 512)[selA], inv2[dst][selA], 512)
    nch2b, base2b, C2b, offs2b, ws2b, avb2b, a2ib, a2vb = _chunk_edges_banded(
        key2[selB], NG, tabrow[selB] - (NG - 1) * NCORES * 512,
        (loc % 512)[selB], inv2[dst][selB], 512)

    # --- node features ---
    xtab = x.astype(np.float16)                           # [N, 512] gather table
    xt = np.zeros((NCORES, IN, NPAD), np.float16)         # feature-major local x
    for c in range(NCORES):
        xt[c, :, :NLOC] = x[c * NLOC:(c + 1) * NLOC].T
    # partition-major: xtP[c][p, g*2048 + kc*512 + f] = xt[c][kc*128+p, g*512+f]
    xtP = np.ascontiguousarray(
        xt.reshape(NCORES, 4, P, NG, 512).transpose(0, 2, 3, 1, 4)
        .reshape(NCORES, P, NG * 2048))

    # --- weight folding (BN eval folded into layer-2 weights) ---
    f64 = np.float64
    s = np.asarray(gamma, f64) / np.sqrt(np.asarray(run_var, f64) + BN_EPS)
    t = np.asarray(beta, f64) - np.asarray(run_mean, f64) * s

    # partition-major weightT: w[k][p, kc*512 + j] = W^T[k][kc*128+p, j]
    def _pmaj_w(W5):
        wt = np.asarray(W5, np.float32).transpose(0, 2, 1).astype(np.float16)
        return np.ascontiguousarray(
            wt.reshape(NREL, 4, P, HID).transpose(0, 2, 1, 3)
            .reshape(NREL, P, 4 * HID))
    wlt = _pmaj_w(Wl5)
    wrt = _pmaj_w(Wr5)

    Wtab = np.concatenate([np.asarray(Wmu_l, f64), np.asarray(Wlv_l, f64)], 0)
    Wself = np.concatenate([np.asarray(Wmu_r, f64), np.asarray(Wlv_r, f64)], 0)
    Wall = np.concatenate([Wtab * s[None, :], Wself * s[None, :]], 0)  # [1024, 2560]
    # partition-major, tab half then self half:
    # wallt[p, side*10240 + r*512 + j] = Wall.T[r*128+p, side*512 + j]
    wt = Wall.T.astype(np.float16).reshape(20, P, 2, 512)
    wallt = np.ascontiguousarray(
        wt.transpose(2, 1, 0, 3).reshape(2, P, 20 * 512)
        .transpose(1, 0, 2).reshape(P, 20 * 1024))

    tW = (Wtab @ t).astype(np.float32)                                  # [512]
    bself = (Wself @ t + np.concatenate(
        [np.asarray(bmu, f64), np.asarray(blv, f64)])).astype(np.float32)

    # bias tiles, laid out [128, n] so a column is a per-partition scalar
    blb = np.ascontiguousarray(
        np.asarray(bl5, np.float32).reshape(NREL * 4, P).T)   # [128, 20]
    twb = np.ascontiguousarray(tW.reshape(4, P).T)            # [128, 4]
    bsb = np.ascontiguousarray(bself.reshape(4, P).T)         # [128, 4]

    meta = (tuple(nch1), tuple(base1), C1, tuple(offs1), tuple(ws1),
            tuple(avb1), tuple(nch2), tuple(base2), C2, tuple(offs2),
            tuple(ws2), tuple(avb2), tuple(nch2b), tuple(base2b), C2b,
            tuple(offs2b), tuple(ws2b), tuple(avb2b))
    in_maps = []
    for c in range(NCORES):
        in_maps.append({
            "xtab": xtab, "xt": xtP[c],
            "a1i": a1i[c], "a1v": a1v[c],
            "a2i": a2i[c], "a2v": a2v[c],
            "a2ib": a2ib[c], "a2vb": a2vb[c],
            "wlt": wlt, "wrt": wrt, "wallt": wallt,
            "blb": blb, "twb": twb, "bsb": bsb,
        })
    return meta, in_maps


# ----------------------------------------------------------------------------
# Device kernel
# ----------------------------------------------------------------------------

def _build(meta):
    import concourse.bacc as bacc
    import concourse.bass as bass
    import concourse.tile as tile
    import concourse.mybir as mybir
    from concourse.masks import make_identity

    (nch1, base1, C1, offs1, ws1, avb1,
     nch2, base2, C2, offs2, ws2, avb2,
     nch2b, base2b, C2b, offs2b, ws2b, avb2b) = meta
    nch1 = np.asarray(nch1).reshape(NG, NREL)
    base1 = np.asarray(base1).reshape(NG, NREL)
    offs1 = np.asarray(offs1)
    ws1 = np.asarray(ws1)
    avb1 = np.asarray(avb1)
    nch2 = np.asarray(nch2)
    base2 = np.asarray(base2)
    offs2 = np.asarray(offs2)
    ws2 = np.asarray(ws2)
    avb2 = np.asarray(avb2)
    nch2b = np.asarray(nch2b)
    base2b = np.asarray(base2b)
    offs2b = np.asarray(offs2b)
    ws2b = np.asarray(ws2b)
    avb2b = np.asarray(avb2b)
    # max av-tile widths per cell (layer 1) / per group (layer 2)
    A1MAX = int(max(avb1[base1[gg, k] + nch1[gg, k]] - avb1[base1[gg, k]]
                    for gg in range(NG) for k in range(NREL)))
    A2MAX = int(max(avb2[base2[gg] + nch2[gg]] - avb2[base2[gg]]
                    for gg in range(NG)))
    A2BMAX = int(max(avb2b[base2b[gg] + nch2b[gg]] - avb2b[base2b[gg]]
                     for gg in range(NG)))

    f16, f32, i32 = mybir.dt.float16, mybir.dt.float32, mybir.dt.int32
    RELU = mybir.ActivationFunctionType.Relu
    IDENT = mybir.ActivationFunctionType.Identity
    COPY = mybir.ActivationFunctionType.Copy

    nc = bacc.Bacc("TRN2", target_bir_lowering=False, debug=False,
                   num_devices=NCORES)

    xtab_t = nc.dram_tensor("xtab", [N, IN], f16, kind="ExternalInput")
    xt_t = nc.dram_tensor("xt", [P, NG * 2048], f16, kind="ExternalInput")
    a1i_t = nc.dram_tensor("a1i", [P, C1], i32, kind="ExternalInput")
    a1v_t = nc.dram_tensor("a1v", [P, int(avb1[-1])], f16,
                           kind="ExternalInput")
    a2i_t = nc.dram_tensor("a2i", [P, C2], i32, kind="ExternalInput")
    a2v_t = nc.dram_tensor("a2v", [P, int(avb2[-1])], f16,
                           kind="ExternalInput")
    a2ib_t = nc.dram_tensor("a2ib", [P, C2b], i32, kind="ExternalInput")
    a2vb_t = nc.dram_tensor("a2vb", [P, int(avb2b[-1])], f16,
                            kind="ExternalInput")
    wlt_t = nc.dram_tensor("wlt", [NREL, P, 4 * HID], f16, kind="ExternalInput")
    wrt_t = nc.dram_tensor("wrt", [NREL, P, 4 * HID], f16, kind="ExternalInput")
    wallt_t = nc.dram_tensor("wallt", [P, 20 * 1024], f16, kind="ExternalInput")
    blb_t = nc.dram_tensor("blb", [P, NREL * 4], f32, kind="ExternalInput")
    twb_t = nc.dram_tensor("twb", [P, 4], f32, kind="ExternalInput")
    bsb_t = nc.dram_tensor("bsb", [P, 4], f32, kind="ExternalInput")
    out_t = nc.dram_tensor("out", [P, NG * 2048], f32, kind="ExternalOutput")

    hrelu = nc.dram_tensor("hrelu", [P, NG * 20 * 512], f16, kind="Internal")
    warm_t = nc.dram_tensor("warm", [P, 512], f16, kind="Internal")
    ag_in = nc.dram_tensor("ag_in", [NPAD, 512], f16, kind="Internal")
    ag_tabA = nc.dram_tensor("ag_tabA", [(NG - 1) * NCORES * 512, 512], f16,
                             kind="Internal", addr_space="Shared")
    ag_tabB = nc.dram_tensor("ag_tabB", [NCORES * 512, 512], f16,
                             kind="Internal", addr_space="Shared")

    with tile.TileContext(nc) as tc:
        with (
            tc.tile_pool(name="constp", bufs=1) as constp,
            tc.tile_pool(name="resp", bufs=1) as resp,
            tc.tile_pool(name="iop", bufs=3) as iop,
            tc.tile_pool(name="gthp", bufs=18) as gthp,
            tc.tile_pool(name="rtp", bufs=6) as rtp,
            tc.tile_pool(name="actp", bufs=2) as actp,
            tc.tile_pool(name="psum", bufs=1, space="PSUM") as pp,
        ):
            # ---- constants / resident tiles ----
            ident = constp.tile([P, P], f16, name="ident", tag="ident")
            make_identity(nc, ident[:])
            # small bias tiles on the vector DMA queue so the sync queue's
            # first transfer is cell 0's one-hot values
            blb_sb = constp.tile([P, NREL * 4], f32, name="blb_sb", tag="blb")
            nc.scalar.dma_start(out=blb_sb[:], in_=blb_t.ap())
            twb_sb = constp.tile([P, 4], f32, name="twb_sb", tag="twb")
            nc.scalar.dma_start(out=twb_sb[:], in_=twb_t.ap())
            bsb_sb = constp.tile([P, 4], f32, name="bsb_sb", tag="bsb")
            nc.scalar.dma_start(out=bsb_sb[:], in_=bsb_t.ap())

            # index tiles via gpsimd's own SWDGE queue so the first gathers
            # don't wait on sync-ring semaphore-lane chains
            idx1_sb = resp.tile([P, C1], i32, name="idx1_sb", tag="idx1")
            nc.gpsimd.dma_start(out=idx1_sb[:], in_=a1i_t.ap())
            idx2_sb = resp.tile([P, C2], i32, name="idx2_sb", tag="idx2")
            nc.gpsimd.dma_start(out=idx2_sb[:], in_=a2i_t.ap())
            idx2b_sb = resp.tile([P, C2b], i32, name="idx2b_sb", tag="idx2b")
            nc.gpsimd.dma_start(out=idx2b_sb[:], in_=a2ib_t.ap())

            # resident weights / local features
            xt_sb = resp.tile([P, NG * 2048], f16, name="xt_sb", tag="xt")
            wl_sb = resp.tile([P, NREL * 2048], f16, name="wl_sb", tag="wl")
            wr_sb = resp.tile([P, NREL * 2048], f16, name="wr_sb", tag="wr")
            wall_sb = resp.tile([P, 20 * 1024], f16, name="wall_sb", tag="wall")

            def wall_chunk(r, colhalf):
                side, ch = colhalf // 4, colhalf % 4
                o = side * 10240 + r * 512 + ch * P
                return wall_sb[:, o:o + P]

            # first-needed data first: xt group 0, wl/wr k=0
            nc.scalar.dma_start(out=xt_sb[:, 0:2048], in_=xt_t.ap()[:, 0:2048])
            nc.scalar.dma_start(out=wl_sb[:, 0:2048], in_=wlt_t.ap()[0])
            nc.scalar.dma_start(out=wr_sb[:, 0:2048], in_=wrt_t.ap()[0])

            # ---- gather streams: one 128-row chunk per indirect DMA ----
            gth_tiles = {}

            def gather_flush(idx_sb, table, total, c0, c1, kind):
                """Emit gather DMAs covering chunk range [c0, c1)."""
                for c in range(c0, min(c1, total)):
                    if (kind, c) in gth_tiles:
                        continue
                    t = gthp.tile([P, 512], f16, name=f"g_{kind}_{c}",
                                  tag="gth")
                    nc.gpsimd.indirect_dma_start(
                        out=t[:], out_offset=None,
                        in_=table.ap(),
                        in_offset=bass.IndirectOffsetOnAxis(
                            ap=idx_sb[:, c:c + 1], axis=0))
                    gth_tiles[(kind, c)] = t

            def gth_slice(kind, c, cc):
                return gth_tiles[(kind, c)][:, cc * P:(cc + 1) * P]

            # ---- PE warm-up while the first DMAs land ----
            wu = constp.tile([P, 512], f16, name="wu", tag="wu")
            nc.vector.memset(wu[:], 0.0)
            wu_ps = [pp.tile([P, 512], f32, space="PSUM", name=f"wu_ps{i}",
                             tag="h", bufs=3) for i in range(2)]
            for i in range(16):
                nc.tensor.matmul(out=wu_ps[i % 2][:], lhsT=wu[:, 0:P],
                                 rhs=wu[:], start=(i < 2), stop=(i >= 14))
            nc.vector.tensor_copy(out=wu[:], in_=wu_ps[0][:])
            nc.scalar.dma_start(out=warm_t.ap(), in_=wu[:])

            # =================================================================
            # Phase 1 (+2a): per group: L1 aggregation + dense per relation,
            # then stacked tab-side projection, PE transpose, per-group
            # AllGather.  Aggregation runs one cell ahead of dense.
            # =================================================================
            def emit_ag(g):
                out = (ag_tabB.ap() if g == NG - 1 else
                       ag_tabA.ap()[g * NCORES * 512:(g + 1) * NCORES * 512, :])
                nc.gpsimd.collective_compute(
                    "AllGather", mybir.AluOpType.bypass,
                    replica_groups=[list(range(NCORES))],
                    ins=[ag_in.ap()[g * 512:(g + 1) * 512, :]],
                    outs=[out])

            cells = [(gg, k) for gg in range(NG) for k in range(NREL)]

            def agg_cell(gg, k):
                """Aggregation MMs for cell (gg, k) -> mean_sb tiles (f16).

                Two passes over the cell's chunks (cc 0/1 then cc 2/3), each
                into its own 2-bank PSUM tile: with bufs=2 on the tag, cell
                j+1's pass A reuses the buffer that cell j's pass A already
                evacuated, so the PE never waits on an evacuation.
                """
                cb = int(base1[gg, k])
                ctot = int(nch1[gg, k])
                gather_flush(idx1_sb, xtab_t, C1, cb, cb + ctot, "x")
                # prefetch the next two cells' gathers too
                gather_flush(idx1_sb, xtab_t, C1, cb + ctot,
                             min(cb + ctot + 10, C1), "x")
                ab = int(avb1[cb])
                aw = int(avb1[cb + ctot]) - ab
                av = iop.tile([P, A1MAX], f16, name=f"a1_{gg}_{k}", tag="av")
                nc.sync.dma_start(
                    out=av[:, :aw], in_=a1v_t.ap()[:, ab:ab + aw])
                mean_sb = [None] * 4
                for half in range(2):
                    mps = pp.tile([P, 1024], f32, space="PSUM",
                                  name=f"agg_{gg}_{k}_{half}", tag="mean",
                                  bufs=2)
                    for ci in range(ctot):
                        c = cb + ci
                        off, w_ = int(offs1[c]), int(ws1[c])
                        ao = int(avb1[c]) - ab
                        for ch in range(2):
                            cc = half * 2 + ch
                            nc.tensor.matmul(
                                out=mps[:, ch * 512 + off:ch * 512 + off + w_],
                                lhsT=gth_slice("x", c, cc),
                                rhs=av[:, ao:ao + w_],
                                start=(ci == 0), stop=(ci == ctot - 1))
                    for ch in range(2):
                        cc = half * 2 + ch
                        m = actp.tile([P, 512], f16, name=f"mean_{gg}_{k}_{cc}",
                                      tag=f"mean{cc}")
                        if ch == 0:
                            nc.vector.tensor_copy(
                                out=m[:], in_=mps[:, ch * 512:(ch + 1) * 512])
                        else:
                            nc.scalar.activation(
                                out=m[:], in_=mps[:, ch * 512:(ch + 1) * 512],
                                func=COPY)
                        mean_sb[cc] = m
                return mean_sb

            def dense_cell(gg, k, mean_sb):
                """h = relu(Wl@mean + Wr@x + b) -> rt tile, spilled to DRAM.

                mc processed in pairs with the two h-tag PSUM banks
                alternating per matmul, so consecutive matmuls never chain
                into the same bank (same-bank accumulation serializes the
                fill/drain and costs ~+200ns per matmul).
                """
                rt = rtp.tile([P, 2048], f16, name=f"relu_{gg}_{k}", tag="rt")
                for mp in range(2):
                    hps = [pp.tile([P, 512], f32, space="PSUM",
                                   name=f"h_{gg}_{k}_{mp}_{i}", tag="h",
                                   bufs=3) for i in range(2)]
                    for kc in range(4):
                        for i in range(2):
                            mc = mp * 2 + i
                            nc.tensor.matmul(
                                out=hps[i][:],
                                lhsT=wl_sb[:, k * 2048 + kc * 512 + mc * P:
                                           k * 2048 + kc * 512 + (mc + 1) * P],
                                rhs=mean_sb[kc][:],
                                start=(kc == 0), stop=False)
                    for kc in range(4):
                        for i in range(2):
                            mc = mp * 2 + i
                            nc.tensor.matmul(
                                out=hps[i][:],
                                lhsT=wr_sb[:, k * 2048 + kc * 512 + mc * P:
                                           k * 2048 + kc * 512 + (mc + 1) * P],
                                rhs=xt_sb[:, gg * 2048 + kc * 512:
                                          gg * 2048 + (kc + 1) * 512],
                                start=False, stop=(kc == 3))
                    for i in range(2):
                        mc = mp * 2 + i
                        b = blb_sb[:, k * 4 + mc:k * 4 + mc + 1]
                        if i == 0:
                            nc.scalar.activation(
                                out=rt[:, mc * 512:(mc + 1) * 512],
                                in_=hps[i][:], func=RELU, bias=b)
                        else:
                            nc.vector.tensor_scalar(
                                out=rt[:, mc * 512:(mc + 1) * 512],
                                in0=hps[i][:], scalar1=b, scalar2=0.0,
                                op0=mybir.AluOpType.add,
                                op1=mybir.AluOpType.max)
                if gg < NG - 1:
                    nc.sync.dma_start(
                        out=hrelu.ap()[:, (gg * 20 + k * 4) * 512:
                                       (gg * 20 + k * 4 + 4) * 512],
                        in_=rt[:])
                return rt

            def proj_pair(mp, side, rts, outs, bias_sb):
                """Two interleaved 20-chunk projection chains (mc = 2*mp,
                2*mp+1) alternating the two h-tag PSUM banks per matmul."""
                hps = [pp.tile([P, 512], f32, space="PSUM",
                               name=f"pj_{side}_{mp}_{i}", tag="h", bufs=3)
                       for i in range(2)]
                for r in range(20):
                    for i in range(2):
                        nc.tensor.matmul(
                            out=hps[i][:],
                            lhsT=wall_chunk(r, side * 4 + mp * 2 + i),
                            rhs=rts[r // 4][:, (r % 4) * 512:(r % 4 + 1) * 512],
                            start=(r == 0), stop=(r == 19))
                for i in range(2):
                    mc = mp * 2 + i
                    b = bias_sb[:, mc:mc + 1]
                    tile_, c0 = outs[i]
                    if i == 0:
                        nc.scalar.activation(
                            out=tile_[:, c0:c0 + 512], in_=hps[i][:],
                            func=IDENT, bias=b)
                    else:
                        nc.vector.tensor_scalar(
                            out=tile_[:, c0:c0 + 512], in0=hps[i][:],
                            scalar1=b, scalar2=None, op0=mybir.AluOpType.add)

            def tab_proj(gg, rts):
                """Aggregated-side stacked projection + transpose + AG input."""
                tab_sb = [actp.tile([P, 512], f16, name=f"tab_{gg}_{mc}",
                                    tag=f"tab{mc}", bufs=1) for mc in range(4)]
                for mp in range(2):
                    proj_pair(mp, 0, rts,
                              [(tab_sb[mp * 2], 0), (tab_sb[mp * 2 + 1], 0)],
                              twb_sb)
                agin = actp.tile([P, 2048], f16, name=f"agin_{gg}", tag="agin",
                                 bufs=1)
                for ns in range(4):
                    tr_ps = pp.tile([P, 1024], f16, space="PSUM",
                                    name=f"tr_{gg}_{ns}", tag="tr", bufs=1)
                    for mc in range(4):
                        nc.tensor.transpose(
                            out=tr_ps[:, mc * P:(mc + 1) * P],
                            in_=tab_sb[mc][:, ns * P:(ns + 1) * P],
                            identity=ident[:])
                    nc.vector.tensor_copy(
                        out=agin[:, ns * 512:(ns + 1) * 512],
                        in_=tr_ps[:, 0:512])
                nc.scalar.dma_start(
                    out=ag_in.ap()[gg * 512:(gg + 1) * 512, :]
                        .rearrange("(ns p) f -> p ns f", p=P),
                    in_=agin[:].rearrange("p (ns f) -> p ns f", f=512))

            sf_all = {}

            def selfproj(gg, rts):
                sf = actp.tile([P, 2048], f16, name=f"selfr_{gg}",
                               tag=f"sf{gg}", bufs=1)
                for mp in range(2):
                    proj_pair(mp, 1, rts,
                              [(sf, mp * 1024), (sf, mp * 1024 + 512)],
                              bsb_sb)
                sf_all[gg] = sf

            # ---- phase-1 main loop with one-cell-ahead aggregation ----
            pending_ag = None
            mean_next = agg_cell(*cells[0])
            for j, (gg, k) in enumerate(cells):
                mean_cur = mean_next
                # deferred DMAs for data needed soon, staggered to keep HBM
                # bandwidth free for the gather/one-hot stream early on
                if 0 <= j <= 3:
                    kk = j + 1
                    nc.scalar.dma_start(
                        out=wl_sb[:, kk * 2048:(kk + 1) * 2048],
                        in_=wlt_t.ap()[kk])
                    nc.scalar.dma_start(
                        out=wr_sb[:, kk * 2048:(kk + 1) * 2048],
                        in_=wrt_t.ap()[kk])
                if j == 3:
                    nc.scalar.dma_start(out=wall_sb[:, :10 * 1024],
                                        in_=wallt_t.ap()[:, :10 * 1024])
                if j == 10:
                    nc.scalar.dma_start(out=wall_sb[:, 10 * 1024:],
                                        in_=wallt_t.ap()[:, 10 * 1024:])
                if k == 3 and gg + 1 < NG:
                    nc.scalar.dma_start(
                        out=xt_sb[:, (gg + 1) * 2048:(gg + 2) * 2048],
                        in_=xt_t.ap()[:, (gg + 1) * 2048:(gg + 2) * 2048])
                if j + 1 < len(cells):
                    mean_next = agg_cell(*cells[j + 1])
                    if pending_ag is not None:
                        emit_ag(pending_ag)
                        pending_ag = None
                rt = dense_cell(gg, k, mean_cur)
                if k == 0:
                    rts = []
                rts.append(rt)
                if k == NREL - 1:
                    tab_proj(gg, rts)
                    if gg == NG - 1:
                        emit_ag(gg)   # last AG: nothing left to protect
                    else:
                        pending_ag = gg

            # =================================================================
            # Phase 3: self-side projections (cover the last AllGather and
            # pace the layer-2 gather stream) interleaved with the layer-2
            # banded aggregation.  The aggregation runs in two passes:
            # pass A (sources in groups 0..3, table A — gathers overlap the
            # last AllGather's flight) accumulates into the sf tiles; pass B
            # (group-4 sources, table B) finishes and emits the output.
            # =================================================================
            self_order = [NG - 1] + list(range(NG - 1))
            for gg in self_order:
                cb = int(base2[gg])
                gather_flush(idx2_sb, ag_tabA, C2, cb, cb + int(nch2[gg]),
                             "t")
            for gg in self_order:
                cb = int(base2b[gg])
                gather_flush(idx2b_sb, ag_tabB, C2b, cb,
                             cb + int(nch2b[gg]), "u")

            def reload_rts(gg):
                out = []
                for k in range(NREL):
                    rt = rtp.tile([P, 2048], f16, name=f"hrB_{gg}_{k}",
                                  tag="rt")
                    nc.sync.dma_start(
                        out=rt[:],
                        in_=hrelu.ap()[:, (gg * 20 + k * 4) * 512:
                                       (gg * 20 + k * 4 + 4) * 512])
                    out.append(rt)
                return out

            def l2pass(gg, kind, idx_cols, nchv, basev, offsv, wsv, avbv,
                       av_t, avmax):
                cb = int(basev[gg])
                ctot = int(nchv[gg])
                ab = int(avbv[cb])
                aw = int(avbv[cb + ctot]) - ab
                av2 = iop.tile([P, avmax], f16, name=f"a2{kind}_{gg}",
                               tag=f"av2{kind}", bufs=2)
                nc.sync.dma_start(
                    out=av2[:, :aw], in_=av_t.ap()[:, ab:ab + aw])
                m2 = [pp.tile([P, 1024], f32, space="PSUM",
                              name=f"m2{kind}_{gg}_{h}", tag="mean", bufs=2)
                      for h in range(2)]
                for ci in range(ctot):
                    c = cb + ci
                    off, w_ = int(offsv[c]), int(wsv[c])
                    ao = int(avbv[c]) - ab
                    for cc in range(4):
                        nc.tensor.matmul(
                            out=m2[cc // 2][:, (cc % 2) * 512 + off:
                                            (cc % 2) * 512 + off + w_],
                            lhsT=gth_slice(kind, c, cc),
                            rhs=av2[:, ao:ao + w_],
                            start=(ci == 0), stop=(ci == ctot - 1))
                return m2

            def l2aggA(gg):
                sf = sf_all[gg]
                m2 = l2pass(gg, "t", idx2_sb, nch2, base2, offs2, ws2, avb2,
                            a2v_t, A2MAX)
                for mc in range(4):
                    nc.vector.tensor_tensor(
                        out=sf[:, mc * 512:(mc + 1) * 512],
                        in0=m2[mc // 2][:, (mc % 2) * 512:(mc % 2 + 1) * 512],
                        in1=sf[:, mc * 512:(mc + 1) * 512],
                        op=mybir.AluOpType.add)

            def l2aggB(gg):
                sf = sf_all[gg]
                m2 = l2pass(gg, "u", idx2b_sb, nch2b, base2b, offs2b, ws2b,
                            avb2b, a2vb_t, A2BMAX)
                for mc in range(4):
                    ob = actp.tile([P, 512], f32, name=f"out_{gg}_{mc}",
                                   tag="outsb", bufs=2)
                    nc.vector.tensor_tensor(
                        out=ob[:],
                        in0=m2[mc // 2][:, (mc % 2) * 512:(mc % 2 + 1) * 512],
                        in1=sf[:, mc * 512:(mc + 1) * 512],
                        op=mybir.AluOpType.add)
                    nc.scalar.dma_start(
                        out=out_t.ap()[:, gg * 2048 + mc * 512:
                                       gg * 2048 + (mc + 1) * 512],
                        in_=ob[:])

            # interleaved schedule matching the gather emission order
            selfproj(NG - 1, rts)
            selfproj(0, reload_rts(0))
            selfproj(1, reload_rts(1))
            l2aggA(NG - 1)
            for gg in range(2, NG - 1):
                selfproj(gg, reload_rts(gg))
                l2aggA(gg - 2)
            for gg in range(NG - 3, NG - 1):
                l2aggA(gg)
            for gg in [NG - 1] + list(range(NG - 1)):
                l2aggB(gg)

    nc.compile()
    return nc


# ----------------------------------------------------------------------------
# Entry point
# ----------------------------------------------------------------------------

_CACHE = {}


def build_and_run(inputs, trace=False, trace_kwargs=None):
    from concourse import bass_utils

    meta, in_maps = _preprocess(**inputs)
    if meta not in _CACHE:
        _CACHE[meta] = _build(meta)
    nc = _CACHE[meta]
    res = bass_utils.run_bass_kernel_spmd(
        nc, in_maps, core_ids=list(range(NCORES)),
        trace=trace, **(trace_kwargs or {}))

    mu = np.empty((N, OUT), np.float32)
    lv = np.empty((N, OUT), np.float32)
    for c in range(NCORES):
        raw = res.results[c]["out"]            # [128, NG*2048] fp32 p-major
        blk = raw.reshape(P, NG, 4, 512).transpose(2, 0, 1, 3).reshape(
            512, NG * 512)                     # [512 ch, 2560 nodes]
        mu[c * NLOC:(c + 1) * NLOC] = blk[0:OUT, :NLOC].T
        lv[c * NLOC:(c + 1) * NLOC] = blk[OUT:2 * OUT, :NLOC].T
    return (mu, lv), res


def kernel(**inputs):
    out, _ = build_and_run(inputs, trace=False)
    return out


# revision 50
# speedup vs baseline: 1.0804x; 1.0804x over previous
"""Trainium2 Bass kernel for a 2-layer relational GraphSAGE VGAE encoder.

Contract: kernel(**inputs) takes the FULL unsharded inputs (as produced by
setup_inputs()) and returns the full (mu, logvar) tuple.

Strategy (8 NeuronCores, SPMD single NEFF):
  - Nodes block-sharded: core c owns nodes [c*2500, (c+1)*2500), padded to 2560.
  - Edges partitioned by destination-node owner and sorted by destination so
    each <=128-edge chunk scatters into a single 64-node window: the one-hot
    matmuls run at N=64 instead of N=512 (4-8x less PE time than group-wide
    one-hots).
  - Gathers are batched 4 chunks per indirect DMA ([128, 4] index columns)
    to amortize the SWDGE fixed cost.
  - Feature-major [channels, nodes] fp16 everywhere, fp32 PSUM accumulation.
  - BatchNorm (eval) folded into the layer-2 weights on the host; layer 2 has
    a single stacked projection weight [2560 -> 1024] covering mu_l|lv_l
    (aggregated side) and mu_r|lv_r (self side).
  - The aggregated-side projections are PE-transposed to node-major and
    AllGather'd (fp16) per node group so the collectives overlap phase 1;
    the self-side matmuls (phase 3a) cover the last AllGather's tail.
  - Dense layers restructured mc-outer so h accumulates in 1-bank PSUM tiles;
    aggregation for cell j+1 is emitted ahead of dense for cell j so the PE
    never waits on PSUM->SBUF evacuations (which are split across the Vector
    and Scalar engines).
"""
import sys

sys.path.insert(0, "/opt/trn_rl_repo")

import numpy as np

NCORES = 8
N = 20000
E = 100000
IN = 512
HID = 512
CAT = 2560
OUT = 256
BN_EPS = 1e-5

NLOC = N // NCORES          # 2500
NPAD = 2560                 # 20 * 128, 5 * 512
NG = NPAD // 512            # 5 node groups of 512 per core
NREL = 5
P = 128
TABROWS = NCORES * NPAD     # 20480

WIN = 64                    # scatter window width (nodes per one-hot matmul)
WPG = 512 // WIN            # windows per group (8)
NW = NPAD // WIN            # windows per core (40)


# ----------------------------------------------------------------------------
# Host-side preprocessing: sharding, edge chunking, weight folding
# ----------------------------------------------------------------------------

def _chunk_edges_banded(key, ncells, src_vals, col, val, width):
    """Group edges by per-core cell, sort by destination column within the
    cell, and cut into 128-edge chunks.  Each chunk scatters into a narrow
    column band [off, off+W) shared across cores (compile-time constants).
    Bands are forced contiguous and to cover [0, width) so every PSUM
    element gets written.

    Returns: nch [ncells], base [ncells], Ctot, offs [Ctot], ws [Ctot],
             avb [Ctot+1] av column base per chunk,
             idxT [NCORES, 128, Ctot] int32,
             avP [NCORES, 128, avb[-1]] f16 (partition-major one-hot values)
    """
    counts = np.bincount(key, minlength=NCORES * ncells).reshape(NCORES, ncells)
    nch = np.maximum((counts + P - 1) // P, 1).max(axis=0)  # [ncells]
    base = np.concatenate([[0], np.cumsum(nch)[:-1]])
    Ctot = int(nch.sum())

    order = np.lexsort((col, key))
    ks = key[order]
    cols = col[order]
    first_of_run = np.r_[True, ks[1:] != ks[:-1]]
    run_starts = np.flatnonzero(first_of_run)
    run_id = np.cumsum(first_of_run) - 1
    pos = np.arange(len(ks)) - run_starts[run_id]

    core_s = ks // ncells
    cell_s = ks % ncells
    chunk_s = base[cell_s] + pos // P
    row_s = pos % P

    # per-chunk min/max destination column across cores
    lo = np.full(Ctot, width, np.int64)
    hi = np.zeros(Ctot, np.int64)
    np.minimum.at(lo, chunk_s, cols)
    np.maximum.at(hi, chunk_s, cols + 1)
    offs = np.zeros(Ctot, np.int64)
    ws = np.zeros(Ctot, np.int64)
    for cell in range(ncells):
        b, n = base[cell], nch[cell]
        end = 0
        for i in range(b, b + n):
            offs[i] = 0 if i == b else min(lo[i], end)
            end = max(hi[i], offs[i] + 1)
            ws[i] = end - offs[i]
        ws[b + n - 1] = width - offs[b + n - 1]
    avb = np.concatenate([[0], np.cumsum(ws)])

    idxT = np.zeros((NCORES, P, Ctot), np.int32)
    avP = np.zeros((NCORES, P, int(avb[-1])), np.float16)
    idxT[core_s, row_s, chunk_s] = src_vals[order]
    avP[core_s, row_s, avb[chunk_s] + cols - offs[chunk_s]] = val[order]
    return nch, base, Ctot, offs, ws, avb, idxT, avP


def _chunk_edges(key, ncells, src_vals, col, val):
    """Group edges by per-core cell, chunk each cell into 128-edge chunks.

    key: [E] int = core * ncells + cell   (cell < ncells)
    src_vals: [E] int32 gather row index for each edge
    col: [E] int in [0, WIN) one-hot column (dst position within window)
    val: [E] f32 one-hot value (1/cnt)

    Returns: nch [ncells] shared chunk counts (max over cores, >=1),
             base [ncells] chunk base offsets, Ctot,
             idxT [NCORES, 128, Ctot] int32,
             avP [NCORES, 128, Ctot*WIN] f16 (partition-major one-hot values)
    """
    counts = np.bincount(key, minlength=NCORES * ncells).reshape(NCORES, ncells)
    nch = np.maximum((counts + P - 1) // P, 1).max(axis=0)  # [ncells]
    base = np.concatenate([[0], np.cumsum(nch)[:-1]])
    Ctot = int(nch.sum())

    order = np.argsort(key, kind="stable")
    ks = key[order]
    first_of_run = np.r_[True, ks[1:] != ks[:-1]]
    run_starts = np.flatnonzero(first_of_run)
    run_id = np.cumsum(first_of_run) - 1
    pos = np.arange(len(ks)) - run_starts[run_id]

    core_s = ks // ncells
    cell_s = ks % ncells
    chunk_s = base[cell_s] + pos // P
    row_s = pos % P

    idxT = np.zeros((NCORES, P, Ctot), np.int32)
    av = np.zeros((NCORES, Ctot, P, WIN), np.float16)
    idxT[core_s, row_s, chunk_s] = src_vals[order]
    av[core_s, chunk_s, row_s, col[order]] = val[order]
    avP = np.ascontiguousarray(
        av.transpose(0, 2, 1, 3).reshape(NCORES, P, Ctot * WIN))
    return nch, base, Ctot, idxT, avP


def _preprocess(x, edge_index, edge_attr, Wl5, Wr5, bl5,
                Wmu_l, Wmu_r, bmu, Wlv_l, Wlv_r, blv,
                gamma, beta, run_mean, run_var):
    x = np.asarray(x, np.float32)
    src = np.asarray(edge_index[0], np.int64)
    dst = np.asarray(edge_index[1], np.int64)
    rel = np.asarray(edge_attr, np.int64)

    # --- per-node inverse degree ---
    cnt1 = np.bincount(rel * N + dst, minlength=NREL * N).reshape(NREL, N)
    inv1 = 1.0 / np.maximum(cnt1, 1.0)
    cnt2 = np.bincount(dst, minlength=N)
    inv2 = 1.0 / np.maximum(cnt2, 1.0)

    core = dst // NLOC
    loc = dst % NLOC
    g = loc // 512
    w_in_g = (loc // WIN) % WPG
    col = loc % WIN

    # layer-1 cells ordered (group, rel): banded chunks sorted by dst column
    cell1 = g * NREL + rel
    key1 = core * (NG * NREL) + cell1
    nch1, base1, C1, offs1, ws1, avb1, a1i, a1v = _chunk_edges_banded(
        key1, NG * NREL, src.astype(np.int32), loc % 512, inv1[rel, dst], 512)

    # layer-2 cells ordered (group, window); gather rows from the all-gathered
    # table whose layout is [g][core][col]: row = g*8*512 + core*512 + col.
    src_loc = src % NLOC
    tabrow = ((src_loc // 512) * (NCORES * 512) + (src // NLOC) * 512
              + src_loc % 512).astype(np.int32)
    # split by source group: rows < 16384 are in table A (groups 0-3,
    # complete after AG3); group-4 rows live in table B (after AG4)
    isB = tabrow >= (NG - 1) * NCORES * 512
    selA, selB = ~isB, isB
    key2 = core * NG + g
    nch2, base2, C2, offs2, ws2, avb2, a2i, a2v = _chunk_edges_banded(
        key2[selA], NG, tabrow[selA], (loc % 512)[selA], inv2[dst][selA], 512)
    nch2b, base2b, C2b, offs2b, ws2b, avb2b, a2ib, a2vb = _chunk_edges_banded(
        key2[selB], NG, tabrow[selB] - (NG - 1) * NCORES * 512,
        (loc % 512)[selB], inv2[dst][selB], 512)

    # --- node features ---
    xtab = x.astype(np.float16)                           # [N, 512] gather table
    xt = np.zeros((NCORES, IN, NPAD), np.float16)         # feature-major local x
    for c in range(NCORES):
        xt[c, :, :NLOC] = x[c * NLOC:(c + 1) * NLOC].T
    # partition-major: xtP[c][p, g*2048 + kc*512 + f] = xt[c][kc*128+p, g*512+f]
    xtP = np.ascontiguousarray(
        xt.reshape(NCORES, 4, P, NG, 512).transpose(0, 2, 3, 1, 4)
        .reshape(NCORES, P, NG * 2048))

    # --- weight folding (BN eval folded into layer-2 weights) ---
    f64 = np.float64
    s = np.asarray(gamma, f64) / np.sqrt(np.asarray(run_var, f64) + BN_EPS)
    t = np.asarray(beta, f64) - np.asarray(run_mean, f64) * s

    # partition-major weightT: w[k][p, kc*512 + j] = W^T[k][kc*128+p, j]
    def _pmaj_w(W5):
        wt = np.asarray(W5, np.float32).transpose(0, 2, 1).astype(np.float16)
        return np.ascontiguousarray(
            wt.reshape(NREL, 4, P, HID).transpose(0, 2, 1, 3)
            .reshape(NREL, P, 4 * HID))
    wlt = _pmaj_w(Wl5)
    wrt = _pmaj_w(Wr5)

    Wtab = np.concatenate([np.asarray(Wmu_l, f64), np.asarray(Wlv_l, f64)], 0)
    Wself = np.concatenate([np.asarray(Wmu_r, f64), np.asarray(Wlv_r, f64)], 0)
    Wall = np.concatenate([Wtab * s[None, :], Wself * s[None, :]], 0)  # [1024, 2560]
    # partition-major, tab half then self half:
    # wallt[p, side*10240 + r*512 + j] = Wall.T[r*128+p, side*512 + j]
    wt = Wall.T.astype(np.float16).reshape(20, P, 2, 512)
    wallt = np.ascontiguousarray(
        wt.transpose(2, 1, 0, 3).reshape(2, P, 20 * 512)
        .transpose(1, 0, 2).reshape(P, 20 * 1024))

    tW = (Wtab @ t).astype(np.float32)                                  # [512]
    bself = (Wself @ t + np.concatenate(
        [np.asarray(bmu, f64), np.asarray(blv, f64)])).astype(np.float32)

    # bias tiles, laid out [128, n] so a column is a per-partition scalar
    blb = np.ascontiguousarray(
        np.asarray(bl5, np.float32).reshape(NREL * 4, P).T)   # [128, 20]
    twb = np.ascontiguousarray(tW.reshape(4, P).T)            # [128, 4]
    bsb = np.ascontiguousarray(bself.reshape(4, P).T)         # [128, 4]

    meta = (tuple(nch1), tuple(base1), C1, tuple(offs1), tuple(ws1),
            tuple(avb1), tuple(nch2), tuple(base2), C2, tuple(offs2),
            tuple(ws2), tuple(avb2), tuple(nch2b), tuple(base2b), C2b,
            tuple(offs2b), tuple(ws2b), tuple(avb2b))
    in_maps = []
    for c in range(NCORES):
        in_maps.append({
            "xtab": xtab, "xt": xtP[c],
            "a1i": a1i[c], "a1v": a1v[c],
            "a2i": a2i[c], "a2v": a2v[c],
            "a2ib": a2ib[c], "a2vb": a2vb[c],
            "wlt": wlt, "wrt": wrt, "wallt": wallt,
            "blb": blb, "twb": twb, "bsb": bsb,
        })
    return meta, in_maps


# ----------------------------------------------------------------------------
# Device kernel
# ----------------------------------------------------------------------------

def _build(meta):
    import concourse.bacc as bacc
    import concourse.bass as bass
    import concourse.tile as tile
    import concourse.mybir as mybir
    from concourse.masks import make_identity

    (nch1, base1, C1, offs1, ws1, avb1,
     nch2, base2, C2, offs2, ws2, avb2,
     nch2b, base2b, C2b, offs2b, ws2b, avb2b) = meta
    nch1 = np.asarray(nch1).reshape(NG, NREL)
    base1 = np.asarray(base1).reshape(NG, NREL)
    offs1 = np.asarray(offs1)
    ws1 = np.asarray(ws1)
    avb1 = np.asarray(avb1)
    nch2 = np.asarray(nch2)
    base2 = np.asarray(base2)
    offs2 = np.asarray(offs2)
    ws2 = np.asarray(ws2)
    avb2 = np.asarray(avb2)
    nch2b = np.asarray(nch2b)
    base2b = np.asarray(base2b)
    offs2b = np.asarray(offs2b)
    ws2b = np.asarray(ws2b)
    avb2b = np.asarray(avb2b)
    # max av-tile widths per cell (layer 1) / per group (layer 2)
    A1MAX = int(max(avb1[base1[gg, k] + nch1[gg, k]] - avb1[base1[gg, k]]
                    for gg in range(NG) for k in range(NREL)))
    A2MAX = int(max(avb2[base2[gg] + nch2[gg]] - avb2[base2[gg]]
                    for gg in range(NG)))
    A2BMAX = int(max(avb2b[base2b[gg] + nch2b[gg]] - avb2b[base2b[gg]]
                     for gg in range(NG)))

    f16, f32, i32 = mybir.dt.float16, mybir.dt.float32, mybir.dt.int32
    RELU = mybir.ActivationFunctionType.Relu
    IDENT = mybir.ActivationFunctionType.Identity
    COPY = mybir.ActivationFunctionType.Copy

    nc = bacc.Bacc("TRN2", target_bir_lowering=False, debug=False,
                   num_devices=NCORES)

    xtab_t = nc.dram_tensor("xtab", [N, IN], f16, kind="ExternalInput")
    xt_t = nc.dram_tensor("xt", [P, NG * 2048], f16, kind="ExternalInput")
    a1i_t = nc.dram_tensor("a1i", [P, C1], i32, kind="ExternalInput")
    a1v_t = nc.dram_tensor("a1v", [P, int(avb1[-1])], f16,
                           kind="ExternalInput")
    a2i_t = nc.dram_tensor("a2i", [P, C2], i32, kind="ExternalInput")
    a2v_t = nc.dram_tensor("a2v", [P, int(avb2[-1])], f16,
                           kind="ExternalInput")
    a2ib_t = nc.dram_tensor("a2ib", [P, C2b], i32, kind="ExternalInput")
    a2vb_t = nc.dram_tensor("a2vb", [P, int(avb2b[-1])], f16,
                            kind="ExternalInput")
    wlt_t = nc.dram_tensor("wlt", [NREL, P, 4 * HID], f16, kind="ExternalInput")
    wrt_t = nc.dram_tensor("wrt", [NREL, P, 4 * HID], f16, kind="ExternalInput")
    wallt_t = nc.dram_tensor("wallt", [P, 20 * 1024], f16, kind="ExternalInput")
    blb_t = nc.dram_tensor("blb", [P, NREL * 4], f32, kind="ExternalInput")
    twb_t = nc.dram_tensor("twb", [P, 4], f32, kind="ExternalInput")
    bsb_t = nc.dram_tensor("bsb", [P, 4], f32, kind="ExternalInput")
    out_t = nc.dram_tensor("out", [P, NG * 2048], f32, kind="ExternalOutput")

    hrelu = nc.dram_tensor("hrelu", [P, NG * 20 * 512], f16, kind="Internal")
    warm_t = nc.dram_tensor("warm", [P, 512], f16, kind="Internal")
    ag_in = nc.dram_tensor("ag_in", [NPAD, 512], f16, kind="Internal")
    ag_tabA = nc.dram_tensor("ag_tabA", [(NG - 1) * NCORES * 512, 512], f16,
                             kind="Internal", addr_space="Shared")
    ag_tabB = nc.dram_tensor("ag_tabB", [NCORES * 512, 512], f16,
                             kind="Internal", addr_space="Shared")

    with tile.TileContext(nc) as tc:
        with (
            tc.tile_pool(name="constp", bufs=1) as constp,
            tc.tile_pool(name="resp", bufs=1) as resp,
            tc.tile_pool(name="iop", bufs=3) as iop,
            tc.tile_pool(name="gthp", bufs=24) as gthp,
            tc.tile_pool(name="rtp", bufs=5) as rtp,
            tc.tile_pool(name="actp", bufs=2) as actp,
            tc.tile_pool(name="psum", bufs=1, space="PSUM") as pp,
        ):
            # ---- constants / resident tiles ----
            ident = constp.tile([P, P], f16, name="ident", tag="ident")
            make_identity(nc, ident[:])
            # small bias tiles on the vector DMA queue so the sync queue's
            # first transfer is cell 0's one-hot values
            blb_sb = constp.tile([P, NREL * 4], f32, name="blb_sb", tag="blb")
            nc.scalar.dma_start(out=blb_sb[:], in_=blb_t.ap())
            twb_sb = constp.tile([P, 4], f32, name="twb_sb", tag="twb")
            nc.scalar.dma_start(out=twb_sb[:], in_=twb_t.ap())
            bsb_sb = constp.tile([P, 4], f32, name="bsb_sb", tag="bsb")
            nc.scalar.dma_start(out=bsb_sb[:], in_=bsb_t.ap())

            # index tiles via gpsimd's own SWDGE queue so the first gathers
            # don't wait on sync-ring semaphore-lane chains
            idx1_sb = resp.tile([P, C1], i32, name="idx1_sb", tag="idx1")
            nc.gpsimd.dma_start(out=idx1_sb[:], in_=a1i_t.ap())
            idx2_sb = resp.tile([P, C2], i32, name="idx2_sb", tag="idx2")
            nc.gpsimd.dma_start(out=idx2_sb[:], in_=a2i_t.ap())
            idx2b_sb = resp.tile([P, C2b], i32, name="idx2b_sb", tag="idx2b")
            nc.gpsimd.dma_start(out=idx2b_sb[:], in_=a2ib_t.ap())

            # resident weights / local features
            xt_sb = resp.tile([P, NG * 2048], f16, name="xt_sb", tag="xt")
            wl_sb = resp.tile([P, NREL * 2048], f16, name="wl_sb", tag="wl")
            wr_sb = resp.tile([P, NREL * 2048], f16, name="wr_sb", tag="wr")
            wall_sb = resp.tile([P, 20 * 1024], f16, name="wall_sb", tag="wall")

            def wall_chunk(r, colhalf):
                side, ch = colhalf // 4, colhalf % 4
                o = side * 10240 + r * 512 + ch * P
                return wall_sb[:, o:o + P]

            # first-needed data first: xt group 0, wl/wr k=0
            nc.scalar.dma_start(out=xt_sb[:, 0:2048], in_=xt_t.ap()[:, 0:2048])
            nc.scalar.dma_start(out=wl_sb[:, 0:2048], in_=wlt_t.ap()[0])
            nc.scalar.dma_start(out=wr_sb[:, 0:2048], in_=wrt_t.ap()[0])

            # ---- gather streams: one 128-row chunk per indirect DMA ----
            gth_tiles = {}

            def gather_flush(idx_sb, table, total, c0, c1, kind):
                """Emit gather DMAs covering chunk range [c0, c1)."""
                for c in range(c0, min(c1, total)):
                    if (kind, c) in gth_tiles:
                        continue
                    t = gthp.tile([P, 512], f16, name=f"g_{kind}_{c}",
                                  tag="gth")
                    nc.gpsimd.indirect_dma_start(
                        out=t[:], out_offset=None,
                        in_=table.ap(),
                        in_offset=bass.IndirectOffsetOnAxis(
                            ap=idx_sb[:, c:c + 1], axis=0))
                    gth_tiles[(kind, c)] = t

            def gth_slice(kind, c, cc):
                return gth_tiles[(kind, c)][:, cc * P:(cc + 1) * P]

            # ---- PE warm-up while the first DMAs land ----
            wu = constp.tile([P, 512], f16, name="wu", tag="wu")
            nc.vector.memset(wu[:], 0.0)
            wu_ps = [pp.tile([P, 512], f32, space="PSUM", name=f"wu_ps{i}",
                             tag="h", bufs=3) for i in range(2)]
            for i in range(16):
                nc.tensor.matmul(out=wu_ps[i % 2][:], lhsT=wu[:, 0:P],
                                 rhs=wu[:], start=(i < 2), stop=(i >= 14))
            nc.vector.tensor_copy(out=wu[:], in_=wu_ps[0][:])
            nc.scalar.dma_start(out=warm_t.ap(), in_=wu[:])

            # =================================================================
            # Phase 1 (+2a): per group: L1 aggregation + dense per relation,
            # then stacked tab-side projection, PE transpose, per-group
            # AllGather.  Aggregation runs one cell ahead of dense.
            # =================================================================
            def emit_ag(g):
                out = (ag_tabB.ap() if g == NG - 1 else
                       ag_tabA.ap()[g * NCORES * 512:(g + 1) * NCORES * 512, :])
                nc.gpsimd.collective_compute(
                    "AllGather", mybir.AluOpType.bypass,
                    replica_groups=[list(range(NCORES))],
                    ins=[ag_in.ap()[g * 512:(g + 1) * 512, :]],
                    outs=[out])

            cells = [(gg, k) for gg in range(NG) for k in range(NREL)]

            def agg_cell(gg, k):
                """Aggregation MMs for cell (gg, k) -> mean_sb tiles (f16).

                Two passes over the cell's chunks (cc 0/1 then cc 2/3), each
                into its own 2-bank PSUM tile: with bufs=2 on the tag, cell
                j+1's pass A reuses the buffer that cell j's pass A already
                evacuated, so the PE never waits on an evacuation.
                """
                cb = int(base1[gg, k])
                ctot = int(nch1[gg, k])
                gather_flush(idx1_sb, xtab_t, C1, cb, cb + ctot, "x")
                # prefetch the next two cells' gathers too
                gather_flush(idx1_sb, xtab_t, C1, cb + ctot,
                             min(cb + ctot + 10, C1), "x")
                ab = int(avb1[cb])
                aw = int(avb1[cb + ctot]) - ab
                av = iop.tile([P, A1MAX], f16, name=f"a1_{gg}_{k}", tag="av")
                nc.sync.dma_start(
                    out=av[:, :aw], in_=a1v_t.ap()[:, ab:ab + aw])
                mean_sb = [None] * 4
                for half in range(2):
                    mps = pp.tile([P, 1024], f32, space="PSUM",
                                  name=f"agg_{gg}_{k}_{half}", tag="mean",
                                  bufs=2)
                    for ci in range(ctot):
                        c = cb + ci
                        off, w_ = int(offs1[c]), int(ws1[c])
                        ao = int(avb1[c]) - ab
                        for ch in range(2):
                            cc = half * 2 + ch
                            nc.tensor.matmul(
                                out=mps[:, ch * 512 + off:ch * 512 + off + w_],
                                lhsT=gth_slice("x", c, cc),
                                rhs=av[:, ao:ao + w_],
                                start=(ci == 0), stop=(ci == ctot - 1))
                    for ch in range(2):
                        cc = half * 2 + ch
                        m = actp.tile([P, 512], f16, name=f"mean_{gg}_{k}_{cc}",
                                      tag=f"mean{cc}")
                        if ch == 0:
                            nc.vector.tensor_copy(
                                out=m[:], in_=mps[:, ch * 512:(ch + 1) * 512])
                        else:
                            nc.scalar.activation(
                                out=m[:], in_=mps[:, ch * 512:(ch + 1) * 512],
                                func=COPY)
                        mean_sb[cc] = m
                return mean_sb

            def dense_cell(gg, k, mean_sb):
                """h = relu(Wl@mean + Wr@x + b) -> rt tile, spilled to DRAM.

                mc processed in pairs with the two h-tag PSUM banks
                alternating per matmul, so consecutive matmuls never chain
                into the same bank (same-bank accumulation serializes the
                fill/drain and costs ~+200ns per matmul).
                """
                rt = rtp.tile([P, 2048], f16, name=f"relu_{gg}_{k}", tag="rt")
                for mp in range(2):
                    hps = [pp.tile([P, 512], f32, space="PSUM",
                                   name=f"h_{gg}_{k}_{mp}_{i}", tag="h",
                                   bufs=3) for i in range(2)]
                    for kc in range(4):
                        for i in range(2):
                            mc = mp * 2 + i
                            nc.tensor.matmul(
                                out=hps[i][:],
                                lhsT=wl_sb[:, k * 2048 + kc * 512 + mc * P:
                                           k * 2048 + kc * 512 + (mc + 1) * P],
                                rhs=mean_sb[kc][:],
                                start=(kc == 0), stop=False)
                    for kc in range(4):
                        for i in range(2):
                            mc = mp * 2 + i
                            nc.tensor.matmul(
                                out=hps[i][:],
                                lhsT=wr_sb[:, k * 2048 + kc * 512 + mc * P:
                                           k * 2048 + kc * 512 + (mc + 1) * P],
                                rhs=xt_sb[:, gg * 2048 + kc * 512:
                                          gg * 2048 + (kc + 1) * 512],
                                start=False, stop=(kc == 3))
                    for i in range(2):
                        mc = mp * 2 + i
                        b = blb_sb[:, k * 4 + mc:k * 4 + mc + 1]
                        if i == 0:
                            nc.scalar.activation(
                                out=rt[:, mc * 512:(mc + 1) * 512],
                                in_=hps[i][:], func=RELU, bias=b)
                        else:
                            nc.vector.tensor_scalar(
                                out=rt[:, mc * 512:(mc + 1) * 512],
                                in0=hps[i][:], scalar1=b, scalar2=0.0,
                                op0=mybir.AluOpType.add,
                                op1=mybir.AluOpType.max)
                if gg < NG - 1:
                    nc.sync.dma_start(
                        out=hrelu.ap()[:, (gg * 20 + k * 4) * 512:
                                       (gg * 20 + k * 4 + 4) * 512],
                        in_=rt[:])
                return rt

            def proj_pair(mp, side, rts, outs, bias_sb):
                """Two interleaved 20-chunk projection chains (mc = 2*mp,
                2*mp+1) alternating the two h-tag PSUM banks per matmul."""
                hps = [pp.tile([P, 512], f32, space="PSUM",
                               name=f"pj_{side}_{mp}_{i}", tag="h", bufs=3)
                       for i in range(2)]
                for r in range(20):
                    for i in range(2):
                        nc.tensor.matmul(
                            out=hps[i][:],
                            lhsT=wall_chunk(r, side * 4 + mp * 2 + i),
                            rhs=rts[r // 4][:, (r % 4) * 512:(r % 4 + 1) * 512],
                            start=(r == 0), stop=(r == 19))
                for i in range(2):
                    mc = mp * 2 + i
                    b = bias_sb[:, mc:mc + 1]
                    tile_, c0 = outs[i]
                    if i == 0:
                        nc.scalar.activation(
                            out=tile_[:, c0:c0 + 512], in_=hps[i][:],
                            func=IDENT, bias=b)
                    else:
                        nc.vector.tensor_scalar(
                            out=tile_[:, c0:c0 + 512], in0=hps[i][:],
                            scalar1=b, scalar2=None, op0=mybir.AluOpType.add)

            def tab_proj(gg, rts):
                """Aggregated-side stacked projection + transpose + AG input."""
                tab_sb = [actp.tile([P, 512], f16, name=f"tab_{gg}_{mc}",
                                    tag=f"tab{mc}", bufs=1) for mc in range(4)]
                for mp in range(2):
                    proj_pair(mp, 0, rts,
                              [(tab_sb[mp * 2], 0), (tab_sb[mp * 2 + 1], 0)],
                              twb_sb)
                agin = actp.tile([P, 2048], f16, name=f"agin_{gg}", tag="agin",
                                 bufs=1)
                for ns in range(4):
                    tr_ps = pp.tile([P, 1024], f16, space="PSUM",
                                    name=f"tr_{gg}_{ns}", tag="tr", bufs=1)
                    for mc in range(4):
                        nc.tensor.transpose(
                            out=tr_ps[:, mc * P:(mc + 1) * P],
                            in_=tab_sb[mc][:, ns * P:(ns + 1) * P],
                            identity=ident[:])
                    nc.vector.tensor_copy(
                        out=agin[:, ns * 512:(ns + 1) * 512],
                        in_=tr_ps[:, 0:512])
                nc.scalar.dma_start(
                    out=ag_in.ap()[gg * 512:(gg + 1) * 512, :]
                        .rearrange("(ns p) f -> p ns f", p=P),
                    in_=agin[:].rearrange("p (ns f) -> p ns f", f=512))

            sf_all = {}

            def selfproj(gg, rts):
                sf = actp.tile([P, 2048], f16, name=f"selfr_{gg}",
                               tag=f"sf{gg}", bufs=1)
                for mp in range(2):
                    proj_pair(mp, 1, rts,
                              [(sf, mp * 1024), (sf, mp * 1024 + 512)],
                              bsb_sb)
                sf_all[gg] = sf

            # ---- phase-1 main loop with one-cell-ahead aggregation ----
            pending_ag = None
            mean_next = agg_cell(*cells[0])
            for j, (gg, k) in enumerate(cells):
                mean_cur = mean_next
                # deferred DMAs for data needed soon, staggered to keep HBM
                # bandwidth free for the gather/one-hot stream early on
                if 0 <= j <= 3:
                    kk = j + 1
                    nc.scalar.dma_start(
                        out=wl_sb[:, kk * 2048:(kk + 1) * 2048],
                        in_=wlt_t.ap()[kk])
                    nc.scalar.dma_start(
                        out=wr_sb[:, kk * 2048:(kk + 1) * 2048],
                        in_=wrt_t.ap()[kk])
                if j == 3:
                    nc.scalar.dma_start(out=wall_sb[:, :10 * 1024],
                                        in_=wallt_t.ap()[:, :10 * 1024])
                if j == 10:
                    nc.scalar.dma_start(out=wall_sb[:, 10 * 1024:],
                                        in_=wallt_t.ap()[:, 10 * 1024:])
                if k == 3 and gg + 1 < NG:
                    nc.scalar.dma_start(
                        out=xt_sb[:, (gg + 1) * 2048:(gg + 2) * 2048],
                        in_=xt_t.ap()[:, (gg + 1) * 2048:(gg + 2) * 2048])
                if j + 1 < len(cells):
                    mean_next = agg_cell(*cells[j + 1])
                    if pending_ag is not None:
                        emit_ag(pending_ag)
                        pending_ag = None
                rt = dense_cell(gg, k, mean_cur)
                if k == 0:
                    rts = []
                rts.append(rt)
                if k == NREL - 1:
                    tab_proj(gg, rts)
                    if gg == NG - 1:
                        emit_ag(gg)   # last AG: nothing left to protect
                    else:
                        pending_ag = gg

            # =================================================================
            # Phase 3: self-side projections (cover the last AllGather and
            # pace the layer-2 gather stream) interleaved with the layer-2
            # banded aggregation.  The aggregation runs in two passes:
            # pass A (sources in groups 0..3, table A — gathers overlap the
            # last AllGather's flight) accumulates into the sf tiles; pass B
            # (group-4 sources, table B) finishes and emits the output.
            # =================================================================
            self_order = [NG - 1] + list(range(NG - 1))
            for gg in self_order:
                cb = int(base2[gg])
                gather_flush(idx2_sb, ag_tabA, C2, cb, cb + int(nch2[gg]),
                             "t")
            for gg in self_order:
                cb = int(base2b[gg])
                gather_flush(idx2b_sb, ag_tabB, C2b, cb,
                             cb + int(nch2b[gg]), "u")

            def reload_rts(gg):
                out = []
                for k in range(NREL):
                    rt = rtp.tile([P, 2048], f16, name=f"hrB_{gg}_{k}",
                                  tag="rt")
                    nc.sync.dma_start(
                        out=rt[:],
                        in_=hrelu.ap()[:, (gg * 20 + k * 4) * 512:
                                       (gg * 20 + k * 4 + 4) * 512])
                    out.append(rt)
                return out

            def l2pass(gg, kind, idx_cols, nchv, basev, offsv, wsv, avbv,
                       av_t, avmax):
                cb = int(basev[gg])
                ctot = int(nchv[gg])
                ab = int(avbv[cb])
                aw = int(avbv[cb + ctot]) - ab
                av2 = iop.tile([P, avmax], f16, name=f"a2{kind}_{gg}",
                               tag=f"av2{kind}", bufs=2)
                nc.sync.dma_start(
                    out=av2[:, :aw], in_=av_t.ap()[:, ab:ab + aw])
                m2 = [pp.tile([P, 1024], f32, space="PSUM",
                              name=f"m2{kind}_{gg}_{h}", tag="mean", bufs=2)
                      for h in range(2)]
                for ci in range(ctot):
                    c = cb + ci
                    off, w_ = int(offsv[c]), int(wsv[c])
                    ao = int(avbv[c]) - ab
                    for cc in range(4):
                        nc.tensor.matmul(
                            out=m2[cc // 2][:, (cc % 2) * 512 + off:
                                            (cc % 2) * 512 + off + w_],
                            lhsT=gth_slice(kind, c, cc),
                            rhs=av2[:, ao:ao + w_],
                            start=(ci == 0), stop=(ci == ctot - 1))
                return m2

            def l2aggA(gg):
                sf = sf_all[gg]
                m2 = l2pass(gg, "t", idx2_sb, nch2, base2, offs2, ws2, avb2,
                            a2v_t, A2MAX)
                for mc in range(4):
                    nc.vector.tensor_tensor(
                        out=sf[:, mc * 512:(mc + 1) * 512],
                        in0=m2[mc // 2][:, (mc % 2) * 512:(mc % 2 + 1) * 512],
                        in1=sf[:, mc * 512:(mc + 1) * 512],
                        op=mybir.AluOpType.add)

            def l2aggB(gg):
                sf = sf_all[gg]
                m2 = l2pass(gg, "u", idx2b_sb, nch2b, base2b, offs2b, ws2b,
                            avb2b, a2vb_t, A2BMAX)
                for mc in range(4):
                    ob = actp.tile([P, 512], f32, name=f"out_{gg}_{mc}",
                                   tag="outsb", bufs=2)
                    nc.vector.tensor_tensor(
                        out=ob[:],
                        in0=m2[mc // 2][:, (mc % 2) * 512:(mc % 2 + 1) * 512],
                        in1=sf[:, mc * 512:(mc + 1) * 512],
                        op=mybir.AluOpType.add)
                    nc.scalar.dma_start(
                        out=out_t.ap()[:, gg * 2048 + mc * 512:
                                       gg * 2048 + (mc + 1) * 512],
                        in_=ob[:])

            # interleaved schedule matching the gather emission order:
            # each group's A-pass MMs directly follow its self projection so
            # the gather stream's buffers recycle as early as possible
            selfproj(NG - 1, rts)
            l2aggA(NG - 1)
            for gg in range(NG - 1):
                selfproj(gg, reload_rts(gg))
                l2aggA(gg)
            for gg in [NG - 1] + list(range(NG - 1)):
                l2aggB(gg)

    nc.compile()
    return nc


# ----------------------------------------------------------------------------
# Entry point
# ----------------------------------------------------------------------------

_CACHE = {}


def build_and_run(inputs, trace=False, trace_kwargs=None):
    from concourse import bass_utils

    meta, in_maps = _preprocess(**inputs)
    if meta not in _CACHE:
        _CACHE[meta] = _build(meta)
    nc = _CACHE[meta]
    res = bass_utils.run_bass_kernel_spmd(
        nc, in_maps, core_ids=list(range(NCORES)),
        trace=trace, **(trace_kwargs or {}))

    mu = np.empty((N, OUT), np.float32)
    lv = np.empty((N, OUT), np.float32)
    for c in range(NCORES):
        raw = res.results[c]["out"]            # [128, NG*2048] fp32 p-major
        blk = raw.reshape(P, NG, 4, 512).transpose(2, 0, 1, 3).reshape(
            512, NG * 512)                     # [512 ch, 2560 nodes]
        mu[c * NLOC:(c + 1) * NLOC] = blk[0:OUT, :NLOC].T
        lv[c * NLOC:(c + 1) * NLOC] = blk[OUT:2 * OUT, :NLOC].T
    return (mu, lv), res


def kernel(**inputs):
    out, _ = build_and_run(inputs, trace=False)
    return out
